# revision 14
# baseline (speedup 1.0000x reference)
"""Trainium2 Bass kernel for nn_CGRU (spectral-norm linear -> GRU x16 -> per-step
BatchNorm), 8-way model-parallel over the hidden dimension, batch split into two
independently-pipelined halves.

Shapes (hardcoded): B=256, Z=512, H=2048, T=16, 8 cores.

Strategy (v2: swapped matmul orientation)
-----------------------------------------
* Fold:  gi_{t+1} = h_t @ (w_ih @ lin_w).T + (w_ih @ lin_b + b_ih); the r/z
  folded weights merge with w_hh (same sigmoid argument); n keeps i_n and h_n
  separate.  Fused weight per core: [H=2048, 1024 gate cols] (rz 512 | in 256
  | hn 256).
* Swapped orientation: the STATIONARY operand is an h_t chunk [128, 128-batch]
  and W streams as the MOVING operand (N=512 fp16) -> psum is [batch, gates].
  ~36 matmuls per half-step instead of 128 tiny ones; the output projection
  (moving N=64) shares the same loaded stationary h.  Biases ride as a K=1
  matmul with a ones row (no DVE bias adds), emitted before the k-loop so the
  PE has dependency-free work during the AllGather wait.
* Gates come out batch-major, so BatchNorm stats are ones-matmul partition
  reductions, the normalized output needs NO transpose, and the only
  transposes are 2x[128,128] per half-step to return h_new to hidden-major
  for the exchange.
* Each core owns 256 hidden units; an 8-core AllGather reassembles h_t per
  step per batch-half, software-pipelined across the two halves.
"""
import os
import sys
import types
import contextlib
import ctypes

import numpy as np

import concourse.bass as bass
import concourse.bacc as bacc
import concourse.mybir as mybir
import concourse.tile as tile
from concourse.bass import ts
import concourse.bass_utils as _bu
from concourse.bass_utils import run_bass_kernel_spmd
from concourse.masks import make_identity

if os.environ.get("BASS_LDW_OPT", "0") == "1" and not getattr(_bu, "_ldw_patched", False):
    _orig_run_command = _bu.run_command

    def _run_command_ldw(cmd, *a, **kw):
        cmd = ["--enable-ldw-opt=true" if c == "--enable-ldw-opt=false" else c
               for c in cmd]
        return _orig_run_command(cmd, *a, **kw)

    _bu.run_command = _run_command_ldw
    _bu._ldw_patched = True

f32 = mybir.dt.float32
f32r = mybir.dt.float32r
fp16 = mybir.dt.float16
AF = mybir.ActivationFunctionType
OP = mybir.AluOpType

B, Z, H, T, NC = 256, 512, 2048, 16, 8
BH = B // 2           # 128-column batch half
HS = H // NC          # 256 hidden units per core
GR = 3 * HS           # 768 gate rows per core (r,z,n)
FR = 4 * HS           # 1024 fused gate cols per core (rz 512 | in 256 | hn 256)
ZS = Z // NC          # 64 output features per core
KC = H // 128         # 16 contraction chunks
EPS = 1e-5

# vecs column map ([128, 16] fp32 scratch of per-partition scalars)
U0, LB, FB, IS = 0, 4, 8, 12

LAST_EXEC_NS = [None]
LAST_RESULTS = [None]


def _install_ntff_hook():
    """The agent image lacks antenv.axon_hooks; recreate it so
    run_bass_kernel_spmd(trace=True) can capture NTFF profiles via the
    libaxon_pjrt.so C ABI (same as trn_agent_boot)."""
    try:
        import antenv
    except ImportError:
        return
    if "antenv.axon_hooks" in sys.modules:
        return
    so_path = "/opt/axon/libaxon_pjrt.so"
    if not os.path.exists(so_path):
        return
    lib = ctypes.CDLL(so_path)
    if not hasattr(lib, "axon_start_nrt_profile"):
        return
    lib.axon_start_nrt_profile.argtypes = [ctypes.POINTER(ctypes.c_int64), ctypes.c_size_t]
    lib.axon_start_nrt_profile.restype = ctypes.c_int64
    lib.axon_stop_nrt_profile.argtypes = [ctypes.c_char_p]
    lib.axon_stop_nrt_profile.restype = ctypes.c_int64

    @contextlib.contextmanager
    def _hook(output_dir, device_ids):
        import jax

        jax.devices()
        if device_ids:
            ids = (ctypes.c_int64 * len(device_ids))(*device_ids)
            rc = lib.axon_start_nrt_profile(ids, len(device_ids))
        else:
            rc = lib.axon_start_nrt_profile(None, 0)
        if rc != 0:
            raise RuntimeError(f"axon_start_nrt_profile rc={rc}")
        try:
            yield
        finally:
            n = lib.axon_stop_nrt_profile(str(output_dir).encode())
            print(f"profile: {n} file(s) written to {output_dir}", file=sys.stderr)

    mod = types.ModuleType("antenv.axon_hooks")
    _state = {"hook": _hook}
    mod.set_axon_ntff_profile_hook = lambda h: _state.__setitem__("hook", h)
    mod.get_axon_ntff_profile_hook = lambda: _state["hook"]
    sys.modules["antenv.axon_hooks"] = mod
    antenv.axon_hooks = mod


def build_nc():
    nc = bacc.Bacc("TRN2", target_bir_lowering=False, debug=False, num_devices=NC)

    # ---- I/O ----
    zT_in = nc.dram_tensor("zT", [Z, B], f32, kind="ExternalInput")
    fcw_in = nc.dram_tensor("fc_w", [Z, Z], f32, kind="ExternalInput")
    fcwT_in = nc.dram_tensor("fc_wT", [Z, Z], f32, kind="ExternalInput")
    fcu_in = nc.dram_tensor("fc_u", [Z], f32, kind="ExternalInput")
    fcb_in = nc.dram_tensor("fc_b", [Z], f32, kind="ExternalInput")
    wihT_in = nc.dram_tensor("w_ihT_s", [Z, GR], f32, kind="ExternalInput")
    whhT_in = nc.dram_tensor("w_hhT_s", [H, GR], fp16, kind="ExternalInput")
    linw_in = nc.dram_tensor("lin_w", [Z, H], f32, kind="ExternalInput")
    linwT_in = nc.dram_tensor("lin_wT_s", [H, ZS], fp16, kind="ExternalInput")
    linb_in = nc.dram_tensor("lin_b", [Z], f32, kind="ExternalInput")
    linbs_in = nc.dram_tensor("lin_b_s", [ZS], f32, kind="ExternalInput")
    bih_in = nc.dram_tensor("b_ih_s", [GR], f32, kind="ExternalInput")
    bhh_in = nc.dram_tensor("b_hh_s", [GR], f32, kind="ExternalInput")
    y_out = nc.dram_tensor("y_part", [T, B, ZS], f32, kind="ExternalOutput")

    # per-(step, half) collective bounce buffers (ring of NB per half)
    NB = int(os.environ.get("BASS_CC_BUFS", "4"))
    cc_in = [[nc.dram_tensor(f"cc_in{u}_{t}", [HS, BH], fp16) for t in range(NB)]
             for u in range(2)]
    cc_out = [[nc.dram_tensor(f"cc_out{u}_{t}", [H, BH], fp16, addr_space="Shared")
               for t in range(NB)] for u in range(2)]
    cc_in = [[cc_in[u][t % NB] for t in range(T)] for u in range(2)]
    cc_out = [[cc_out[u][t % NB] for t in range(T)] for u in range(2)]
    WARM = int(os.environ.get("BASS_WARM_SZ", "16"))
    cc_fin = nc.dram_tensor("cc_fin", [HS, B], fp16)
    cc_fout = nc.dram_tensor("cc_fout", [H, B], fp16, addr_space="Shared")
    ccw_in = nc.dram_tensor("ccw_in", [WARM], f32)
    ccw_out = nc.dram_tensor("ccw_out", [NC * WARM], f32, addr_space="Shared")
    rg = [list(range(NC))]

    with tile.TileContext(nc) as tc:
        with tc.tile_pool(name="perm", bufs=1) as perm:
            # fire a tiny AllGather immediately: starts the ncfw rendezvous
            # barrier (~50us) concurrently with the setup DMA/compute.
            nc.gpsimd.collective_compute("AllGather", OP.bypass, replica_groups=rg,
                                         ins=[ccw_in.ap().opt()],
                                         outs=[ccw_out.ap().opt()])
            # ---- persistent SBUF ----
            W_all = perm.tile([128, KC, FR], fp16, name="W_all")
            wihT_sb = perm.tile([128, 4, GR], f32r, name="wihT_sb")
            linwT_sb = perm.tile([128, KC, ZS], fp16, name="linwT_sb")
            h_TB = perm.tile([128, 2, KC, B], fp16, name="h_TB")
            h_new = [perm.tile([128, HS], f32, name=f"h_new{u}") for u in range(2)]
            h_new_x = [perm.tile([128, 2, BH], fp16, name=f"h_new_x{u}") for u in range(2)]
            p_sb = perm.tile([128, 4, B], f32r, name="p_sb")
            vecs = perm.tile([128, 16], f32, name="vecs")
            ones_sb = perm.tile([1, 128], f32, name="ones_sb")
            ones16 = perm.tile([1, 128], fp16, name="ones16")
            ones_col = perm.tile([128, 1], f32, name="ones_col")
            ident = perm.tile([128, 128], f32, name="ident")
            bias_row = perm.tile([1, FR], fp16, name="bias_row")
            lb_sb = perm.tile([ZS, 1], f32, name="lb_sb")
            bih_row = perm.tile([1, GR], f32, name="bih_row")
            bhh_row = perm.tile([1, GR], f32, name="bhh_row")
            ci_row = perm.tile([1, GR], f32, name="ci_row")
            c1_row = perm.tile([1, GR], f32, name="c1_row")
            rep1 = perm.tile([128, GR], f32, name="rep1")
            bhhn_rep = perm.tile([128, HS], f32, name="bhhn_rep")
            ru_sb = [perm.tile([128, 2 * HS], f32, name=f"ru_sb{u}") for u in range(2)]
            d_sb = [perm.tile([128, HS], f32, name=f"d_sb{u}") for u in range(2)]
            e_sb = [perm.tile([128, HS], f32, name=f"e_sb{u}") for u in range(2)]
            xf = perm.tile([ZS, B], f32, name="xf")
            yz = perm.tile([ZS, B], f32, name="yz")
            sqz = perm.tile([ZS, B], f32, name="sqz")
            stz = perm.tile([ZS, 16], f32, name="stz")
            magic_z = perm.tile([ZS, 1], mybir.dt.int32, name="magic_z")
            bn_scr = perm.tile([1, 2 * ZS], f32, name="bn_scr")

            sync = nc.sync

            # persistent PSUM: gate banks + proj accumulators per half
            with (
                tc.tile_pool(name="gpsA", bufs=1, space="PSUM") as gpA,
                tc.tile_pool(name="gpsB", bufs=1, space="PSUM") as gpB,
                tc.tile_pool(name="xps", bufs=1, space="PSUM") as xps,
            ):
                gbA = [gpA.tile([128, 512], f32, name=f"gbA{u}") for u in range(2)]
                gbB = [gpB.tile([128, 512], f32, name=f"gbB{u}") for u in range(2)]
                xpt = xps.tile([128, 2, ZS], f32, name="xpt")
                xp = [xpt[:, u, :] for u in range(2)]
                xz = xps.tile([ZS, B], f32, name="xz")

                # ================= SETUP =================
                with tc.tile_pool(name="setup_sb", bufs=1) as ssb:
                    linw_sb = ssb.tile([128, 4, H], f32r, name="linw_sb")
                    fcw_sb = ssb.tile([128, 4, Z], f32, name="fcw_sb")
                    fcwT_sb = ssb.tile([128, 4, Z], f32r, name="fcwT_sb")
                    zT_sb = ssb.tile([128, 4, B], f32r, name="zT_sb")
                    pre1 = ssb.tile([128, 512], f32, name="pre1")
                    t12 = ssb.tile([128, 8], f32, name="t12")
                    sq8 = ssb.tile([128, 8], f32, name="sq8")
                    nrow = ssb.tile([1, 8], f32, name="nrow")

                    # DMA emission order = step-1 critical path first.
                    nc.scalar.dma_start(fcwT_sb[:], fcwT_in.ap().rearrange("(k p) m -> p k m", p=128).bitcast(f32r))
                    nc.scalar.dma_start(zT_sb[:], zT_in.ap().rearrange("(k p) b -> p k b", p=128).bitcast(f32r))
                    sync.dma_start(wihT_sb[:], wihT_in.ap().rearrange("(k p) c -> p k c", p=128).bitcast(f32r))
                    sync.dma_start(vecs[:, U0:U0 + 4], fcu_in.ap().rearrange("(k p) -> p k", p=128))
                    sync.dma_start(vecs[:, LB:LB + 4], linb_in.ap().rearrange("(k p) -> p k", p=128))
                    sync.dma_start(vecs[:, FB:FB + 4], fcb_in.ap().rearrange("(k p) -> p k", p=128))
                    sync.dma_start(bih_row[:], bih_in.ap().rearrange("(a c) -> a c", a=1))
                    sync.dma_start(bhh_row[:], bhh_in.ap().rearrange("(a c) -> a c", a=1))
                    sync.dma_start(lb_sb[:], linbs_in.ap().rearrange("(z a) -> z a", a=1))
                    sync.dma_start(fcw_sb[:], fcw_in.ap().rearrange("(k p) m -> p k m", p=128))
                    nc.gpsimd.memset(ones_sb[:], 1.0)
                    nc.gpsimd.memset(ones16[:], 1.0)
                    nc.gpsimd.memset(ones_col[:], 1.0)
                    nc.gpsimd.memset(magic_z[:], 0x5f3759df)
                    make_identity(nc, ident[:])
                    whhT_r = whhT_in.ap().rearrange("(k p) c -> p k c", p=128)
                    sync.dma_start(W_all[:, :, 0:2 * HS], whhT_r[:, :, 0:2 * HS])
                    sync.dma_start(W_all[:, :, 3 * HS:4 * HS], whhT_r[:, :, 2 * HS:3 * HS])
                    nc.scalar.dma_start(linw_sb[:], linw_in.ap().rearrange("(k p) m -> p k m", p=128).bitcast(f32r))
                    sync.dma_start(linwT_sb[:], linwT_in.ap().rearrange("(k p) c -> p k c", p=128))

                    # --- step-1 input: p = fc_w @ z.T  (psum [z-chunk, B]) ---
                    # (reuses the idle persistent gate banks as scratch)
                    for m in range(4):
                        pp = (gbA if m < 2 else gbB)[0][:, ts(m % 2, B)]
                        for k in range(4):
                            nc.tensor.matmul(pp, fcwT_sb[:, k, ts(m, 128)], zT_sb[:, k, :],
                                             start=(k == 0), stop=(k == 3))
                        nc.vector.tensor_copy(p_sb[:, m, :], pp)

                    # --- spectral norm: inv_sigma = sqrt(|W.T u|^2 / |W (W.T u)|^2) ---
                    for m in range(4):
                        for k in range(4):
                            nc.tensor.matmul(xp[0][:, m:m + 1], fcw_sb[:, k, ts(m, 128)],
                                             vecs[:, U0 + k:U0 + k + 1],
                                             start=(k == 0), stop=(k == 3))
                    nc.vector.tensor_copy(t12[:, 0:4], xp[0][:, 0:4])
                    for m in range(4):
                        for k in range(4):
                            nc.tensor.matmul(xp[0][:, 4 + m:5 + m], fcwT_sb[:, k, ts(m, 128)].bitcast(f32),
                                             t12[:, k:k + 1],
                                             start=(k == 0), stop=(k == 3))
                    nc.vector.tensor_copy(t12[:, 4:8], xp[0][:, 4:8])
                    nc.vector.tensor_tensor(sq8[:], t12[:], t12[:], OP.mult)
                    nc.tensor.matmul(xp[1][0:1, 0:8], ones_col[:], sq8[:],
                                     start=True, stop=True)
                    nc.vector.tensor_copy(nrow[:], xp[1][0:1, 0:8])
                    nc.vector.tensor_reduce(bn_scr[0:1, 0:1], nrow[0:1, 0:4],
                                            mybir.AxisListType.X, OP.add)   # n1
                    nc.vector.tensor_reduce(bn_scr[0:1, 1:2], nrow[0:1, 4:8],
                                            mybir.AxisListType.X, OP.add)   # n2
                    nc.vector.reciprocal(bn_scr[0:1, 2:3], bn_scr[0:1, 1:2])
                    nc.vector.tensor_tensor(bn_scr[0:1, 3:4], bn_scr[0:1, 0:1],
                                            bn_scr[0:1, 2:3], OP.mult)      # n1/n2
                    nc.scalar.activation(bn_scr[0:1, 4:5], bn_scr[0:1, 3:4], AF.Sqrt)
                    nc.tensor.matmul(xp[0][:, 8:9], ones_sb[:], bn_scr[0:1, 4:5],
                                     start=True, stop=True)
                    nc.vector.tensor_copy(vecs[:, IS:IS + 1], xp[0][:, 8:9])

                    # --- bias rows: ci = lin_b @ w_ih_s.T + b_ih ; c1 = fc_b @ ... ---
                    for (dst, src) in ((ci_row, LB), (c1_row, FB)):
                        for k in range(4):
                            nc.tensor.matmul(gbA[1][0:1, 0:512],
                                             vecs[:, src + k:src + k + 1],
                                             wihT_sb[:, k, 0:512].bitcast(f32),
                                             start=(k == 0), stop=(k == 3))
                            nc.tensor.matmul(gbB[1][0:1, 0:256],
                                             vecs[:, src + k:src + k + 1],
                                             wihT_sb[:, k, 512:768].bitcast(f32),
                                             start=(k == 0), stop=(k == 3))
                        nc.vector.tensor_tensor(dst[0:1, 0:512], gbA[1][0:1, 0:512],
                                                bih_row[0:1, 0:512], OP.add)
                        nc.vector.tensor_tensor(dst[0:1, 512:768], gbB[1][0:1, 0:256],
                                                bih_row[0:1, 512:768], OP.add)
                    # steady bias row [1, FR]: [rz: ci+bhh | in: ci_n | hn: bhh_n]
                    nc.vector.tensor_tensor(bias_row[0:1, 0:2 * HS], ci_row[0:1, 0:2 * HS],
                                            bhh_row[0:1, 0:2 * HS], OP.add)
                    nc.vector.tensor_copy(bias_row[0:1, 2 * HS:3 * HS], ci_row[0:1, 2 * HS:3 * HS])
                    nc.vector.tensor_copy(bias_row[0:1, 3 * HS:4 * HS], bhh_row[0:1, 2 * HS:3 * HS])
                    # step-1 rep tiles: rep1 = bcast(c1 + bhh_rz | c1_n), bhhn_rep
                    nc.vector.tensor_tensor(c1_row[0:1, 0:2 * HS], c1_row[0:1, 0:2 * HS],
                                            bhh_row[0:1, 0:2 * HS], OP.add)
                    nc.tensor.matmul(gbA[0][:], ones_sb[:], c1_row[0:1, 0:512],
                                     start=True, stop=True)
                    nc.vector.tensor_copy(rep1[:, 0:512], gbA[0][:])
                    nc.tensor.matmul(gbB[0][:, 0:256], ones_sb[:], c1_row[0:1, 512:768],
                                     start=True, stop=True)
                    nc.vector.tensor_copy(rep1[:, 512:768], gbB[0][:, 0:256])
                    nc.tensor.matmul(gbB[0][:, 256:512], ones_sb[:], bhh_row[0:1, 512:768],
                                     start=True, stop=True)
                    nc.vector.tensor_copy(bhhn_rep[:], gbB[0][:, 256:512])

                    # ================= STEP 1 (from z, h0 = 0) =================
                    # q = p.T @ w_ih_s.T  (psum [batch, 768]);  gates with h0=0
                    with tc.tile_pool(name="tr1", bufs=2, space="PSUM") as tr1:
                        for u in range(2):
                            qA, qB = gbA[u], gbB[u]
                            for k in range(4):
                                lhs = p_sb[:, k, u * BH:(u + 1) * BH]
                                nc.tensor.matmul(qA[:], lhs, wihT_sb[:, k, 0:512],
                                                 start=(k == 0), stop=(k == 3))
                                nc.tensor.matmul(qB[:, 0:256], lhs, wihT_sb[:, k, 512:768],
                                                 start=(k == 0), stop=(k == 3))
                            # pre_rz = isig*q + rep1 ; r/z = sigmoid
                            nc.vector.scalar_tensor_tensor(pre1[:], qA[:],
                                                           vecs[:, IS:IS + 1],
                                                           rep1[:, 0:512], OP.mult, OP.add)
                            nc.scalar.activation(ru_sb[u][:], pre1[:], AF.Sigmoid)
                            # n = tanh(isig*q_n + c1_n + r*bhh_n)
                            nc.vector.scalar_tensor_tensor(pre1[:, 0:HS], qB[:, 0:256],
                                                           vecs[:, IS:IS + 1],
                                                           rep1[:, 512:768], OP.mult, OP.add)
                            nc.vector.tensor_tensor(pre1[:, HS:2 * HS], ru_sb[u][:, 0:HS],
                                                    bhhn_rep[:], OP.mult)
                            nc.vector.tensor_tensor(pre1[:, 0:HS], pre1[:, 0:HS],
                                                    pre1[:, HS:2 * HS], OP.add)
                            nc.scalar.activation(d_sb[u][:], pre1[:, 0:HS], AF.Tanh)
                            # h1 = (1-u)*n = n - u*n
                            nc.vector.tensor_tensor(e_sb[u][:], ru_sb[u][:, HS:2 * HS],
                                                    d_sb[u][:], OP.mult)
                            nc.vector.tensor_tensor(h_new[u][:], d_sb[u][:],
                                                    e_sb[u][:], OP.subtract)
                            # transpose to hidden-major, cast fp16, exchange
                            for j in range(2):
                                tp = tr1.tile([128, 128], f32, tag="tp")
                                nc.tensor.transpose(tp[:], h_new[u][:, ts(j, 128)], ident[:])
                                nc.scalar.activation(h_new_x[u][:, j, :], tp[:], AF.Copy)
                            sync.dma_start(cc_in[u][0].ap()
                                           .rearrange("(j p) b -> p j b", p=128),
                                           h_new_x[u][:, :, :])
                            nc.gpsimd.collective_compute("AllGather", OP.bypass,
                                                         replica_groups=rg,
                                                         ins=[cc_in[u][0].ap().opt()],
                                                         outs=[cc_out[u][0].ap().opt()])

                        # --- fused input weight: W_comb = lin_w.T @ w_ih_s.T ---
                        # (rides the step-1 AllGather windows; reuses gbA/gbB)
                        for hk in range(KC):
                            crz = gbA[hk % 2]
                            cn = gbB[hk % 2]
                            for kz in range(4):
                                nc.tensor.matmul(crz[:], linw_sb[:, kz, ts(hk, 128)],
                                                 wihT_sb[:, kz, 0:512],
                                                 start=(kz == 0), stop=(kz == 3))
                                nc.tensor.matmul(cn[:, 0:256], linw_sb[:, kz, ts(hk, 128)],
                                                 wihT_sb[:, kz, 512:768],
                                                 start=(kz == 0), stop=(kz == 3))
                            nc.vector.tensor_tensor(W_all[:, hk, 0:512], crz[:],
                                                    W_all[:, hk, 0:512], OP.add)
                            nc.vector.tensor_copy(W_all[:, hk, 512:768], cn[:, 0:256])

                        for u in range(2):
                            cco_r = cc_out[u][0].ap().rearrange("(k p) b -> p k b", p=128)
                            bs = slice(u * BH, (u + 1) * BH)
                            sync.dma_start(h_TB[:, 1, 0:KC // 2, bs], cco_r[:, 0:KC // 2, :])
                            nc.scalar.dma_start(h_TB[:, 1, KC // 2:, bs], cco_r[:, KC // 2:, :])

                # ================= RECURRENCE + OUTPUT =================
                with (
                    tc.tile_pool(name="msc", bufs=1, space="PSUM") as mscp,
                    tc.tile_pool(name="fl", bufs=1, space="PSUM") as flp,
                ):
                    msc = mscp.tile([128, 512], f32, name="msc")
                    fltile = flp.tile([128, 512], f32, name="fltile")
                    NFILL = int(os.environ.get("BASS_NFILL", "0"))

                    def fillers(sb, n):
                        # dependency-free warm-keeping matmuls; output discarded
                        for f in range(n):
                            nc.tensor.matmul(fltile[:], h_TB[:, sb, KC - 1, 0:BH],
                                             W_all[:, f % KC, 0:512],
                                             start=True, stop=True)
                    tp2 = [msc[:, ts(j, 128)] for j in range(2)]   # h transposes
                    TLIM = int(os.environ.get("BASS_T_LIM", str(T)))
                    i32 = mybir.dt.int32

                    def emit_rsqrt(out_ap, v_ap):
                        # 1/sqrt(v) via bit-trick seed + 3 Newton iters (DVE,
                        # no ACT table switch); scratch stz[:, 8:16]
                        eng = nc.vector
                        c = lambda k: stz[0:ZS, 8 + k:9 + k]
                        eng.tensor_scalar(c(0).bitcast(i32), v_ap.bitcast(i32), 1,
                                          None, OP.arith_shift_right)
                        eng.tensor_tensor(c(1).bitcast(i32), magic_z[:],
                                          c(0).bitcast(i32), OP.subtract)
                        eng.tensor_scalar(c(2), v_ap, 0.5, None, OP.mult)
                        ycols = (1, 5, 1)
                        for it in range(3):
                            y = c(ycols[it])
                            eng.tensor_tensor(c(6), y, c(2), OP.mult)
                            eng.tensor_tensor(c(3), c(6), y, OP.mult)
                            eng.tensor_scalar(c(4), c(3), -1.0, 1.5, OP.mult, OP.add)
                            dst = out_ap if it == 2 else c(ycols[it + 1])
                            eng.tensor_tensor(dst, y, c(4), OP.mult)

                    def proj_pass(s, sb):
                        """x_s = lin_w_s @ h_s (+lin_b) in z-major; BatchNorm;
                        transpose to batch-major; store y_out[s-1].  All inputs
                        ready when emitted -> runs inside the AllGather wait."""
                        gs = nc.gpsimd
                        for k in range(KC):
                            nc.tensor.matmul(xz[:], linwT_sb[:, k, :],
                                             h_TB[:, sb, k, :],
                                             start=(k == 0), stop=(k == KC - 1))
                        nc.scalar.activation(xf[:], xz[:], AF.Identity, bias=lb_sb[:])
                        nc.vector.tensor_reduce(stz[:, 0:1], xf[:],
                                                mybir.AxisListType.X, OP.add)
                        gs.tensor_tensor(sqz[:], xf[:], xf[:], OP.mult)
                        nc.vector.tensor_reduce(stz[:, 1:2], sqz[:],
                                                mybir.AxisListType.X, OP.add)
                        gs.tensor_scalar(stz[:, 2:3], stz[:, 0:1], 1.0 / B, None, OP.mult)
                        gs.tensor_scalar(stz[:, 3:4], stz[:, 1:2], 1.0 / B, None, OP.mult)
                        gs.tensor_tensor(stz[:, 4:5], stz[:, 2:3], stz[:, 2:3], OP.mult)
                        gs.tensor_tensor(stz[:, 4:5], stz[:, 3:4], stz[:, 4:5], OP.subtract)
                        gs.tensor_scalar(stz[:, 4:5], stz[:, 4:5], 1.0, EPS,
                                         OP.mult, OP.add)
                        emit_rsqrt(stz[:, 6:7], stz[:, 4:5])                    # rs
                        gs.tensor_tensor(stz[:, 7:8], stz[:, 2:3], stz[:, 6:7], OP.mult)
                        gs.tensor_scalar(stz[:, 7:8], stz[:, 7:8], -1.0, None, OP.mult)
                        nc.scalar.activation(yz[:], xf[:], AF.Identity,
                                             bias=stz[:, 7:8], scale=stz[:, 6:7])
                        # strided-DMA transpose: z-major SBUF -> batch-major DRAM
                        nc.scalar.dma_start(y_out.ap()[s - 1].rearrange("b z -> z b"),
                                            yz[:])

                    for s in range(2, TLIM + 1):  # steps 2..T, h_{s-1} -> h_s
                        for u in range(2):
                            sb = (s - 1) % 2
                            us = slice(u * BH, (u + 1) * BH)
                            # bias rows first (no h dependency -> PE head start)
                            nc.tensor.matmul(gbA[u][:], ones16[:], bias_row[0:1, 0:512],
                                             start=True, stop=False)
                            nc.tensor.matmul(gbB[u][:], ones16[:], bias_row[0:1, 512:1024],
                                             start=True, stop=False)
                            for k in range(KC):
                                lhs = h_TB[:, sb, k, us]
                                nc.tensor.matmul(gbA[u][:], lhs, W_all[:, k, 0:512],
                                                 start=False, stop=(k == KC - 1))
                                nc.tensor.matmul(gbB[u][:], lhs, W_all[:, k, 512:1024],
                                                 start=False, stop=(k == KC - 1))
                            # gates: psum layout gbA=[r|z], gbB=[in|hn] (+bias);
                            # elementwise split DVE (j=0) / GpSimd (j=1)
                            nc.scalar.activation(ru_sb[u][:], gbA[u][:], AF.Sigmoid)
                            for j in range(2):
                                js = slice(j * 128, j * 128 + 128)
                                hs2 = slice(HS + j * 128, HS + j * 128 + 128)
                                # psum-reading ops stay on DVE
                                nc.vector.tensor_tensor(e_sb[u][:, js], gbB[u][:, hs2],
                                                        ru_sb[u][:, js], OP.mult)
                                nc.vector.tensor_tensor(e_sb[u][:, js], e_sb[u][:, js],
                                                        gbB[u][:, js], OP.add)
                                nc.scalar.activation(d_sb[u][:, js], e_sb[u][:, js],
                                                     AF.Tanh)
                            for j, eng in ((0, nc.vector), (1, nc.gpsimd)):
                                js = slice(j * 128, j * 128 + 128)
                                hs2 = slice(HS + j * 128, HS + j * 128 + 128)
                                eng.tensor_tensor(e_sb[u][:, js], h_new[u][:, js],
                                                  d_sb[u][:, js], OP.subtract)
                                eng.tensor_tensor(e_sb[u][:, js], ru_sb[u][:, hs2],
                                                  e_sb[u][:, js], OP.mult)
                                eng.tensor_tensor(h_new[u][:, js], d_sb[u][:, js],
                                                  e_sb[u][:, js], OP.add)
                                nc.tensor.transpose(tp2[j], h_new[u][:, ts(j, 128)],
                                                    ident[:])
                                nc.scalar.activation(h_new_x[u][:, j, :], tp2[j], AF.Copy)
                            if s == TLIM:
                                # last step: both halves ride ONE fused AllGather
                                sync.dma_start(
                                    cc_fin.ap().rearrange("(j p) b -> p j b", p=128)
                                    [:, :, us],
                                    h_new_x[u][:, :, :])
                                if u == 1:
                                    nc.gpsimd.collective_compute(
                                        "AllGather", OP.bypass, replica_groups=rg,
                                        ins=[cc_fin.ap().opt()],
                                        outs=[cc_fout.ap().opt()])
                                    cco_r = cc_fout.ap().rearrange("(k p) b -> p k b", p=128)
                                    for v in range(2):
                                        bs = slice(v * BH, (v + 1) * BH)
                                        sync.dma_start(h_TB[:, s % 2, 0:KC // 2, bs],
                                                       cco_r[:, 0:KC // 2, bs])
                                        nc.scalar.dma_start(h_TB[:, s % 2, KC // 2:, bs],
                                                            cco_r[:, KC // 2:, bs])
                            else:
                                sync.dma_start(cc_in[u][s - 1].ap()
                                               .rearrange("(j p) b -> p j b", p=128),
                                               h_new_x[u][:, :, :])
                                nc.gpsimd.collective_compute(
                                    "AllGather", OP.bypass, replica_groups=rg,
                                    ins=[cc_in[u][s - 1].ap().opt()],
                                    outs=[cc_out[u][s - 1].ap().opt()])
                                cco_r = cc_out[u][s - 1].ap().rearrange("(k p) b -> p k b", p=128)
                                # chunk 0 first so the next kloop resumes ASAP
                                sync.dma_start(h_TB[:, s % 2, 0:2, us], cco_r[:, 0:2, :])
                                nc.scalar.dma_start(h_TB[:, s % 2, 2:6, us], cco_r[:, 2:6, :])
                                sync.dma_start(h_TB[:, s % 2, 6:11, us], cco_r[:, 6:11, :])
                                nc.scalar.dma_start(h_TB[:, s % 2, 11:KC, us], cco_r[:, 11:KC, :])
                            if NFILL:
                                fillers(sb, NFILL)
                        # x_{s-1} output pass rides the AllGather windows
                        proj_pass(s - 1, sb)

                    # tail: x_T
                    proj_pass(TLIM, TLIM % 2)

    nc.compile()
    return nc


_NC_CACHE = [None]


def kernel(z, fc_w, fc_b, fc_u, w_ih, w_hh, b_ih, b_hh, lin_w, lin_b):
    z = np.ascontiguousarray(np.asarray(z, dtype=np.float32))
    fc_w = np.ascontiguousarray(np.asarray(fc_w, dtype=np.float32))
    fc_b = np.asarray(fc_b, dtype=np.float32)
    fc_u = np.asarray(fc_u, dtype=np.float32)
    w_ih = np.asarray(w_ih, dtype=np.float32)
    w_hh = np.asarray(w_hh, dtype=np.float32)
    b_ih = np.asarray(b_ih, dtype=np.float32)
    b_hh = np.asarray(b_hh, dtype=np.float32)
    lin_w = np.asarray(lin_w, dtype=np.float32)
    lin_b = np.asarray(lin_b, dtype=np.float32)

    zT = np.ascontiguousarray(z.T)
    fc_wT = np.ascontiguousarray(fc_w.T)
    lin_wT = np.ascontiguousarray(lin_w.T)
    w_ih4 = w_ih.reshape(3, NC, HS, Z)
    w_hh4 = w_hh.reshape(3, NC, HS, H)
    b_ih3 = b_ih.reshape(3, NC, HS)
    b_hh3 = b_hh.reshape(3, NC, HS)

    in_maps = []
    for c in range(NC):
        wihs = w_ih4[:, c].reshape(GR, Z)
        whhs = w_hh4[:, c].reshape(GR, H)
        in_maps.append({
            "zT": zT,
            "fc_w": fc_w,
            "fc_wT": fc_wT,
            "fc_u": fc_u,
            "fc_b": fc_b,
            "w_ihT_s": np.ascontiguousarray(wihs.T),
            "w_hhT_s": np.ascontiguousarray(whhs.T).astype(np.float16),
            "lin_w": lin_w,
            "lin_wT_s": np.ascontiguousarray(lin_wT[:, c * ZS:(c + 1) * ZS]).astype(np.float16),
            "lin_b": lin_b,
            "lin_b_s": np.ascontiguousarray(lin_b[c * ZS:(c + 1) * ZS]),
            "b_ih_s": np.ascontiguousarray(b_ih3[:, c].reshape(GR)),
            "b_hh_s": np.ascontiguousarray(b_hh3[:, c].reshape(GR)),
        })

    if _NC_CACHE[0] is None:
        _NC_CACHE[0] = build_nc()
    nc = _NC_CACHE[0]

    trace = os.environ.get("BASS_KERNEL_TRACE") == "1"
    if trace:
        _install_ntff_hook()
    res = run_bass_kernel_spmd(nc, in_maps, core_ids=list(range(NC)), trace=trace)
    LAST_EXEC_NS[0] = res.exec_time_ns
    LAST_RESULTS[0] = res

    full = np.empty((T, B, Z), dtype=np.float32)
    for c in range(NC):
        full[:, :, c * ZS:(c + 1) * ZS] = res.results[c]["y_part"]
    return full.transpose(1, 0, 2).reshape(B * T, Z)


# revision 15
# speedup vs baseline: 2.6336x; 2.6336x over previous
"""Trainium2 Bass kernel for nn_CGRU (spectral-norm linear -> GRU x16 -> per-step
BatchNorm), 8-way model-parallel over the hidden dimension, batch split into two
independently-pipelined halves.

Shapes (hardcoded): B=256, Z=512, H=2048, T=16, 8 cores.

Strategy (v2: swapped matmul orientation)
-----------------------------------------
* Fold:  gi_{t+1} = h_t @ (w_ih @ lin_w).T + (w_ih @ lin_b + b_ih); the r/z
  folded weights merge with w_hh (same sigmoid argument); n keeps i_n and h_n
  separate.  Fused weight per core: [H=2048, 1024 gate cols] (rz 512 | in 256
  | hn 256).
* Swapped orientation: the STATIONARY operand is an h_t chunk [128, 128-batch]
  and W streams as the MOVING operand (N=512 fp16) -> psum is [batch, gates].
  ~36 matmuls per half-step instead of 128 tiny ones; the output projection
  (moving N=64) shares the same loaded stationary h.  Biases ride as a K=1
  matmul with a ones row (no DVE bias adds), emitted before the k-loop so the
  PE has dependency-free work during the AllGather wait.
* Gates come out batch-major, so BatchNorm stats are ones-matmul partition
  reductions, the normalized output needs NO transpose, and the only
  transposes are 2x[128,128] per half-step to return h_new to hidden-major
  for the exchange.
* Each core owns 256 hidden units; an 8-core AllGather reassembles h_t per
  step per batch-half, software-pipelined across the two halves.
"""
import os
import sys
import types
import contextlib
import ctypes

import numpy as np

import concourse.bass as bass
import concourse.bacc as bacc
import concourse.mybir as mybir
import concourse.tile as tile
from concourse.bass import ts
import concourse.bass_utils as _bu
from concourse.bass_utils import run_bass_kernel_spmd
from concourse.masks import make_identity

if os.environ.get("BASS_LDW_OPT", "0") == "1" and not getattr(_bu, "_ldw_patched", False):
    _orig_run_command = _bu.run_command

    def _run_command_ldw(cmd, *a, **kw):
        cmd = ["--enable-ldw-opt=true" if c == "--enable-ldw-opt=false" else c
               for c in cmd]
        return _orig_run_command(cmd, *a, **kw)

    _bu.run_command = _run_command_ldw
    _bu._ldw_patched = True

f32 = mybir.dt.float32
f32r = mybir.dt.float32r
fp16 = mybir.dt.float16
AF = mybir.ActivationFunctionType
OP = mybir.AluOpType

B, Z, H, T, NC = 256, 512, 2048, 16, 8
BH = B // 2           # 128-column batch half
HS = H // NC          # 256 hidden units per core
GR = 3 * HS           # 768 gate rows per core (r,z,n)
FR = 4 * HS           # 1024 fused gate cols per core (rz 512 | in 256 | hn 256)
ZS = Z // NC          # 64 output features per core
KC = H // 128         # 16 contraction chunks
EPS = 1e-5

# vecs column map ([128, 16] fp32 scratch of per-partition scalars)
U0, LB, FB, IS = 0, 4, 8, 12

LAST_EXEC_NS = [None]
LAST_RESULTS = [None]


def _install_ntff_hook():
    """The agent image lacks antenv.axon_hooks; recreate it so
    run_bass_kernel_spmd(trace=True) can capture NTFF profiles via the
    libaxon_pjrt.so C ABI (same as trn_agent_boot)."""
    try:
        import antenv
    except ImportError:
        return
    if "antenv.axon_hooks" in sys.modules:
        return
    so_path = "/opt/axon/libaxon_pjrt.so"
    if not os.path.exists(so_path):
        return
    lib = ctypes.CDLL(so_path)
    if not hasattr(lib, "axon_start_nrt_profile"):
        return
    lib.axon_start_nrt_profile.argtypes = [ctypes.POINTER(ctypes.c_int64), ctypes.c_size_t]
    lib.axon_start_nrt_profile.restype = ctypes.c_int64
    lib.axon_stop_nrt_profile.argtypes = [ctypes.c_char_p]
    lib.axon_stop_nrt_profile.restype = ctypes.c_int64

    @contextlib.contextmanager
    def _hook(output_dir, device_ids):
        import jax

        jax.devices()
        if device_ids:
            ids = (ctypes.c_int64 * len(device_ids))(*device_ids)
            rc = lib.axon_start_nrt_profile(ids, len(device_ids))
        else:
            rc = lib.axon_start_nrt_profile(None, 0)
        if rc != 0:
            raise RuntimeError(f"axon_start_nrt_profile rc={rc}")
        try:
            yield
        finally:
            n = lib.axon_stop_nrt_profile(str(output_dir).encode())
            print(f"profile: {n} file(s) written to {output_dir}", file=sys.stderr)

    mod = types.ModuleType("antenv.axon_hooks")
    _state = {"hook": _hook}
    mod.set_axon_ntff_profile_hook = lambda h: _state.__setitem__("hook", h)
    mod.get_axon_ntff_profile_hook = lambda: _state["hook"]
    sys.modules["antenv.axon_hooks"] = mod
    antenv.axon_hooks = mod


def build_nc():
    nc = bacc.Bacc("TRN2", target_bir_lowering=False, debug=False, num_devices=NC)

    # ---- I/O ----
    zT_in = nc.dram_tensor("zT", [Z, B], f32, kind="ExternalInput")
    fcw_in = nc.dram_tensor("fc_w", [Z, Z], f32, kind="ExternalInput")
    fcwT_in = nc.dram_tensor("fc_wT", [Z, Z], f32, kind="ExternalInput")
    fcu_in = nc.dram_tensor("fc_u", [Z], f32, kind="ExternalInput")
    fcb_in = nc.dram_tensor("fc_b", [Z], f32, kind="ExternalInput")
    wihT_in = nc.dram_tensor("w_ihT_s", [Z, GR], f32, kind="ExternalInput")
    whhT_in = nc.dram_tensor("w_hhT_s", [H, GR], fp16, kind="ExternalInput")
    linw_in = nc.dram_tensor("lin_w", [Z, H], f32, kind="ExternalInput")
    linwT_in = nc.dram_tensor("lin_wT_s", [H, ZS], fp16, kind="ExternalInput")
    linb_in = nc.dram_tensor("lin_b", [Z], f32, kind="ExternalInput")
    linbs_in = nc.dram_tensor("lin_b_s", [ZS], f32, kind="ExternalInput")
    bih_in = nc.dram_tensor("b_ih_s", [GR], f32, kind="ExternalInput")
    bhh_in = nc.dram_tensor("b_hh_s", [GR], f32, kind="ExternalInput")
    y_out = nc.dram_tensor("y_part", [T, B, ZS], f32, kind="ExternalOutput")

    # per-(step, half) collective bounce buffers (ring of NB per half)
    NB = int(os.environ.get("BASS_CC_BUFS", "4"))
    cc_in = [[nc.dram_tensor(f"cc_in{u}_{t}", [HS, BH], fp16) for t in range(NB)]
             for u in range(2)]
    cc_out = [[nc.dram_tensor(f"cc_out{u}_{t}", [H, BH], fp16, addr_space="Shared")
               for t in range(NB)] for u in range(2)]
    cc_in = [[cc_in[u][t % NB] for t in range(T)] for u in range(2)]
    cc_out = [[cc_out[u][t % NB] for t in range(T)] for u in range(2)]
    WARM = int(os.environ.get("BASS_WARM_SZ", "16"))
    cc_fin = nc.dram_tensor("cc_fin", [HS, B], fp16)
    cc_fout = nc.dram_tensor("cc_fout", [H, B], fp16, addr_space="Shared")
    ccw_in = nc.dram_tensor("ccw_in", [WARM], f32)
    ccw_out = nc.dram_tensor("ccw_out", [NC * WARM], f32, addr_space="Shared")
    rg = [list(range(NC))]

    with tile.TileContext(nc) as tc:
        with tc.tile_pool(name="perm", bufs=1) as perm:
            # fire a tiny AllGather immediately: starts the ncfw rendezvous
            # barrier (~50us) concurrently with the setup DMA/compute.
            nc.gpsimd.collective_compute("AllGather", OP.bypass, replica_groups=rg,
                                         ins=[ccw_in.ap().opt()],
                                         outs=[ccw_out.ap().opt()])
            # ---- persistent SBUF ----
            W_all = perm.tile([128, KC, FR], fp16, name="W_all")
            wihT_sb = perm.tile([128, 4, GR], f32r, name="wihT_sb")
            linwT_sb = perm.tile([128, KC, ZS], fp16, name="linwT_sb")
            h_TB = perm.tile([128, 2, KC, B], fp16, name="h_TB")
            h_new = [perm.tile([128, HS], f32, name=f"h_new{u}") for u in range(2)]
            h_new_x = [perm.tile([128, 2, BH], fp16, name=f"h_new_x{u}") for u in range(2)]
            p_sb = perm.tile([128, 4, B], f32r, name="p_sb")
            vecs = perm.tile([128, 16], f32, name="vecs")
            ones_sb = perm.tile([1, 128], f32, name="ones_sb")
            ones16 = perm.tile([1, 128], fp16, name="ones16")
            ones_col = perm.tile([128, 1], f32, name="ones_col")
            ident = perm.tile([128, 128], f32, name="ident")
            bias_row = perm.tile([1, FR], fp16, name="bias_row")
            lb_sb = perm.tile([ZS, 1], f32, name="lb_sb")
            bih_row = perm.tile([1, GR], f32, name="bih_row")
            bhh_row = perm.tile([1, GR], f32, name="bhh_row")
            ci_row = perm.tile([1, GR], f32, name="ci_row")
            c1_row = perm.tile([1, GR], f32, name="c1_row")
            rep1 = perm.tile([128, GR], f32, name="rep1")
            bhhn_rep = perm.tile([128, HS], f32, name="bhhn_rep")
            ru_sb = [perm.tile([128, 2 * HS], f32, name=f"ru_sb{u}") for u in range(2)]
            d_sb = [perm.tile([128, HS], f32, name=f"d_sb{u}") for u in range(2)]
            e_sb = [perm.tile([128, HS], f32, name=f"e_sb{u}") for u in range(2)]
            ybm = [perm.tile([128, ZS], f32, name=f"ybm{u}") for u in range(2)]
            xf = perm.tile([ZS, B], f32, name="xf")
            yz = perm.tile([ZS, B], f32, name="yz")
            sqz = perm.tile([ZS, B], f32, name="sqz")
            stz = perm.tile([ZS, 16], f32, name="stz")
            magic_z = perm.tile([ZS, 1], mybir.dt.int32, name="magic_z")
            bn_scr = perm.tile([1, 2 * ZS], f32, name="bn_scr")

            sync = nc.sync

            # persistent PSUM: gate banks + proj accumulators per half
            with (
                tc.tile_pool(name="gpsA", bufs=1, space="PSUM") as gpA,
                tc.tile_pool(name="gpsB", bufs=1, space="PSUM") as gpB,
                tc.tile_pool(name="xps", bufs=1, space="PSUM") as xps,
            ):
                gbA = [gpA.tile([128, 512], f32, name=f"gbA{u}") for u in range(2)]
                gbB = [gpB.tile([128, 512], f32, name=f"gbB{u}") for u in range(2)]
                xpt = xps.tile([128, 2, ZS], f32, name="xpt")
                xp = [xpt[:, u, :] for u in range(2)]
                xz = xps.tile([ZS, B], f32, name="xz")

                # ================= SETUP =================
                with tc.tile_pool(name="setup_sb", bufs=1) as ssb:
                    linw_sb = ssb.tile([128, 4, H], f32r, name="linw_sb")
                    fcw_sb = ssb.tile([128, 4, Z], f32, name="fcw_sb")
                    fcwT_sb = ssb.tile([128, 4, Z], f32r, name="fcwT_sb")
                    zT_sb = ssb.tile([128, 4, B], f32r, name="zT_sb")
                    pre1 = ssb.tile([128, 512], f32, name="pre1")
                    t12 = ssb.tile([128, 8], f32, name="t12")
                    sq8 = ssb.tile([128, 8], f32, name="sq8")
                    nrow = ssb.tile([1, 8], f32, name="nrow")

                    # DMA emission order = step-1 critical path first.
                    nc.scalar.dma_start(fcwT_sb[:], fcwT_in.ap().rearrange("(k p) m -> p k m", p=128).bitcast(f32r))
                    nc.scalar.dma_start(zT_sb[:], zT_in.ap().rearrange("(k p) b -> p k b", p=128).bitcast(f32r))
                    sync.dma_start(wihT_sb[:], wihT_in.ap().rearrange("(k p) c -> p k c", p=128).bitcast(f32r))
                    sync.dma_start(vecs[:, U0:U0 + 4], fcu_in.ap().rearrange("(k p) -> p k", p=128))
                    sync.dma_start(vecs[:, LB:LB + 4], linb_in.ap().rearrange("(k p) -> p k", p=128))
                    sync.dma_start(vecs[:, FB:FB + 4], fcb_in.ap().rearrange("(k p) -> p k", p=128))
                    sync.dma_start(bih_row[:], bih_in.ap().rearrange("(a c) -> a c", a=1))
                    sync.dma_start(bhh_row[:], bhh_in.ap().rearrange("(a c) -> a c", a=1))
                    sync.dma_start(lb_sb[:], linbs_in.ap().rearrange("(z a) -> z a", a=1))
                    sync.dma_start(fcw_sb[:], fcw_in.ap().rearrange("(k p) m -> p k m", p=128))
                    nc.gpsimd.memset(ones_sb[:], 1.0)
                    nc.gpsimd.memset(ones16[:], 1.0)
                    nc.gpsimd.memset(ones_col[:], 1.0)
                    nc.gpsimd.memset(magic_z[:], 0x5f3759df)
                    make_identity(nc, ident[:])
                    whhT_r = whhT_in.ap().rearrange("(k p) c -> p k c", p=128)
                    sync.dma_start(W_all[:, :, 0:2 * HS], whhT_r[:, :, 0:2 * HS])
                    sync.dma_start(W_all[:, :, 3 * HS:4 * HS], whhT_r[:, :, 2 * HS:3 * HS])
                    nc.scalar.dma_start(linw_sb[:], linw_in.ap().rearrange("(k p) m -> p k m", p=128).bitcast(f32r))
                    sync.dma_start(linwT_sb[:], linwT_in.ap().rearrange("(k p) c -> p k c", p=128))

                    # --- step-1 input: p = fc_w @ z.T  (psum [z-chunk, B]) ---
                    # (reuses the idle persistent gate banks as scratch)
                    for m in range(4):
                        pp = (gbA if m < 2 else gbB)[0][:, ts(m % 2, B)]
                        for k in range(4):
                            nc.tensor.matmul(pp, fcwT_sb[:, k, ts(m, 128)], zT_sb[:, k, :],
                                             start=(k == 0), stop=(k == 3))
                        nc.vector.tensor_copy(p_sb[:, m, :], pp)

                    # --- spectral norm: inv_sigma = sqrt(|W.T u|^2 / |W (W.T u)|^2) ---
                    for m in range(4):
                        for k in range(4):
                            nc.tensor.matmul(xp[0][:, m:m + 1], fcw_sb[:, k, ts(m, 128)],
                                             vecs[:, U0 + k:U0 + k + 1],
                                             start=(k == 0), stop=(k == 3))
                    nc.vector.tensor_copy(t12[:, 0:4], xp[0][:, 0:4])
                    for m in range(4):
                        for k in range(4):
                            nc.tensor.matmul(xp[0][:, 4 + m:5 + m], fcwT_sb[:, k, ts(m, 128)].bitcast(f32),
                                             t12[:, k:k + 1],
                                             start=(k == 0), stop=(k == 3))
                    nc.vector.tensor_copy(t12[:, 4:8], xp[0][:, 4:8])
                    nc.vector.tensor_tensor(sq8[:], t12[:], t12[:], OP.mult)
                    nc.tensor.matmul(xp[1][0:1, 0:8], ones_col[:], sq8[:],
                                     start=True, stop=True)
                    nc.vector.tensor_copy(nrow[:], xp[1][0:1, 0:8])
                    nc.vector.tensor_reduce(bn_scr[0:1, 0:1], nrow[0:1, 0:4],
                                            mybir.AxisListType.X, OP.add)   # n1
                    nc.vector.tensor_reduce(bn_scr[0:1, 1:2], nrow[0:1, 4:8],
                                            mybir.AxisListType.X, OP.add)   # n2
                    nc.vector.reciprocal(bn_scr[0:1, 2:3], bn_scr[0:1, 1:2])
                    nc.vector.tensor_tensor(bn_scr[0:1, 3:4], bn_scr[0:1, 0:1],
                                            bn_scr[0:1, 2:3], OP.mult)      # n1/n2
                    nc.scalar.activation(bn_scr[0:1, 4:5], bn_scr[0:1, 3:4], AF.Sqrt)
                    nc.tensor.matmul(xp[0][:, 8:9], ones_sb[:], bn_scr[0:1, 4:5],
                                     start=True, stop=True)
                    nc.vector.tensor_copy(vecs[:, IS:IS + 1], xp[0][:, 8:9])

                    # --- bias rows: ci = lin_b @ w_ih_s.T + b_ih ; c1 = fc_b @ ... ---
                    for (dst, src) in ((ci_row, LB), (c1_row, FB)):
                        for k in range(4):
                            nc.tensor.matmul(gbA[1][0:1, 0:512],
                                             vecs[:, src + k:src + k + 1],
                                             wihT_sb[:, k, 0:512].bitcast(f32),
                                             start=(k == 0), stop=(k == 3))
                            nc.tensor.matmul(gbB[1][0:1, 0:256],
                                             vecs[:, src + k:src + k + 1],
                                             wihT_sb[:, k, 512:768].bitcast(f32),
                                             start=(k == 0), stop=(k == 3))
                        nc.vector.tensor_tensor(dst[0:1, 0:512], gbA[1][0:1, 0:512],
                                                bih_row[0:1, 0:512], OP.add)
                        nc.vector.tensor_tensor(dst[0:1, 512:768], gbB[1][0:1, 0:256],
                                                bih_row[0:1, 512:768], OP.add)
                    # steady bias row [1, FR]: [rz: ci+bhh | in: ci_n | hn: bhh_n]
                    nc.vector.tensor_tensor(bias_row[0:1, 0:2 * HS], ci_row[0:1, 0:2 * HS],
                                            bhh_row[0:1, 0:2 * HS], OP.add)
                    nc.vector.tensor_copy(bias_row[0:1, 2 * HS:3 * HS], ci_row[0:1, 2 * HS:3 * HS])
                    nc.vector.tensor_copy(bias_row[0:1, 3 * HS:4 * HS], bhh_row[0:1, 2 * HS:3 * HS])
                    # step-1 rep tiles: rep1 = bcast(c1 + bhh_rz | c1_n), bhhn_rep
                    nc.vector.tensor_tensor(c1_row[0:1, 0:2 * HS], c1_row[0:1, 0:2 * HS],
                                            bhh_row[0:1, 0:2 * HS], OP.add)
                    nc.tensor.matmul(gbA[0][:], ones_sb[:], c1_row[0:1, 0:512],
                                     start=True, stop=True)
                    nc.vector.tensor_copy(rep1[:, 0:512], gbA[0][:])
                    nc.tensor.matmul(gbB[0][:, 0:256], ones_sb[:], c1_row[0:1, 512:768],
                                     start=True, stop=True)
                    nc.vector.tensor_copy(rep1[:, 512:768], gbB[0][:, 0:256])
                    nc.tensor.matmul(gbB[0][:, 256:512], ones_sb[:], bhh_row[0:1, 512:768],
                                     start=True, stop=True)
                    nc.vector.tensor_copy(bhhn_rep[:], gbB[0][:, 256:512])

                    # ================= STEP 1 (from z, h0 = 0) =================
                    # q = p.T @ w_ih_s.T  (psum [batch, 768]);  gates with h0=0
                    with tc.tile_pool(name="tr1", bufs=2, space="PSUM") as tr1:
                        for u in range(2):
                            qA, qB = gbA[u], gbB[u]
                            for k in range(4):
                                lhs = p_sb[:, k, u * BH:(u + 1) * BH]
                                nc.tensor.matmul(qA[:], lhs, wihT_sb[:, k, 0:512],
                                                 start=(k == 0), stop=(k == 3))
                                nc.tensor.matmul(qB[:, 0:256], lhs, wihT_sb[:, k, 512:768],
                                                 start=(k == 0), stop=(k == 3))
                            # pre_rz = isig*q + rep1 ; r/z = sigmoid
                            nc.vector.scalar_tensor_tensor(pre1[:], qA[:],
                                                           vecs[:, IS:IS + 1],
                                                           rep1[:, 0:512], OP.mult, OP.add)
                            nc.scalar.activation(ru_sb[u][:], pre1[:], AF.Sigmoid)
                            # n = tanh(isig*q_n + c1_n + r*bhh_n)
                            nc.vector.scalar_tensor_tensor(pre1[:, 0:HS], qB[:, 0:256],
                                                           vecs[:, IS:IS + 1],
                                                           rep1[:, 512:768], OP.mult, OP.add)
                            nc.vector.tensor_tensor(pre1[:, HS:2 * HS], ru_sb[u][:, 0:HS],
                                                    bhhn_rep[:], OP.mult)
                            nc.vector.tensor_tensor(pre1[:, 0:HS], pre1[:, 0:HS],
                                                    pre1[:, HS:2 * HS], OP.add)
                            nc.scalar.activation(d_sb[u][:], pre1[:, 0:HS], AF.Tanh)
                            # h1 = (1-u)*n = n - u*n
                            nc.vector.tensor_tensor(e_sb[u][:], ru_sb[u][:, HS:2 * HS],
                                                    d_sb[u][:], OP.mult)
                            nc.vector.tensor_tensor(h_new[u][:], d_sb[u][:],
                                                    e_sb[u][:], OP.subtract)
                            # transpose to hidden-major, cast fp16, exchange
                            for j in range(2):
                                tp = tr1.tile([128, 128], f32, tag="tp")
                                nc.tensor.transpose(tp[:], h_new[u][:, ts(j, 128)], ident[:])
                                nc.scalar.activation(h_new_x[u][:, j, :], tp[:], AF.Copy)
                            sync.dma_start(cc_in[u][0].ap()
                                           .rearrange("(j p) b -> p j b", p=128),
                                           h_new_x[u][:, :, :])
                            nc.gpsimd.collective_compute("AllGather", OP.bypass,
                                                         replica_groups=rg,
                                                         ins=[cc_in[u][0].ap().opt()],
                                                         outs=[cc_out[u][0].ap().opt()])

                        # --- fused input weight: W_comb = lin_w.T @ w_ih_s.T ---
                        # (rides the step-1 AllGather windows; reuses gbA/gbB)
                        for hk in range(KC):
                            crz = gbA[hk % 2]
                            cn = gbB[hk % 2]
                            for kz in range(4):
                                nc.tensor.matmul(crz[:], linw_sb[:, kz, ts(hk, 128)],
                                                 wihT_sb[:, kz, 0:512],
                                                 start=(kz == 0), stop=(kz == 3))
                                nc.tensor.matmul(cn[:, 0:256], linw_sb[:, kz, ts(hk, 128)],
                                                 wihT_sb[:, kz, 512:768],
                                                 start=(kz == 0), stop=(kz == 3))
                            nc.vector.tensor_tensor(W_all[:, hk, 0:512], crz[:],
                                                    W_all[:, hk, 0:512], OP.add)
                            nc.vector.tensor_copy(W_all[:, hk, 512:768], cn[:, 0:256])

                        for u in range(2):
                            cco_r = cc_out[u][0].ap().rearrange("(k p) b -> p k b", p=128)
                            bs = slice(u * BH, (u + 1) * BH)
                            sync.dma_start(h_TB[:, 1, 0:KC // 2, bs], cco_r[:, 0:KC // 2, :])
                            nc.scalar.dma_start(h_TB[:, 1, KC // 2:, bs], cco_r[:, KC // 2:, :])

                # ================= RECURRENCE + OUTPUT =================
                with (
                    tc.tile_pool(name="msc", bufs=1, space="PSUM") as mscp,
                    tc.tile_pool(name="fl", bufs=1, space="PSUM") as flp,
                ):
                    msc = mscp.tile([128, 512], f32, name="msc")
                    fltile = flp.tile([128, 512], f32, name="fltile")
                    NFILL = int(os.environ.get("BASS_NFILL", "0"))

                    def fillers(sb, n):
                        # dependency-free warm-keeping matmuls; output discarded
                        for f in range(n):
                            nc.tensor.matmul(fltile[:], h_TB[:, sb, KC - 1, 0:BH],
                                             W_all[:, f % KC, 0:512],
                                             start=True, stop=True)
                    tp2 = [msc[:, ts(j, 128)] for j in range(2)]   # h transposes
                    typ = [msc[:, 256 + j * 64:256 + (j + 1) * 64] for j in range(2)]
                    TLIM = int(os.environ.get("BASS_T_LIM", str(T)))
                    i32 = mybir.dt.int32

                    def emit_rsqrt(out_ap, v_ap):
                        # 1/sqrt(v) via bit-trick seed + 3 Newton iters (DVE,
                        # no ACT table switch); scratch stz[:, 8:16]
                        eng = nc.vector
                        c = lambda k: stz[0:ZS, 8 + k:9 + k]
                        eng.tensor_scalar(c(0).bitcast(i32), v_ap.bitcast(i32), 1,
                                          None, OP.arith_shift_right)
                        eng.tensor_tensor(c(1).bitcast(i32), magic_z[:],
                                          c(0).bitcast(i32), OP.subtract)
                        eng.tensor_scalar(c(2), v_ap, 0.5, None, OP.mult)
                        ycols = (1, 5, 1)
                        for it in range(3):
                            y = c(ycols[it])
                            eng.tensor_tensor(c(6), y, c(2), OP.mult)
                            eng.tensor_tensor(c(3), c(6), y, OP.mult)
                            eng.tensor_scalar(c(4), c(3), -1.0, 1.5, OP.mult, OP.add)
                            dst = out_ap if it == 2 else c(ycols[it + 1])
                            eng.tensor_tensor(dst, y, c(4), OP.mult)

                    def proj_pass(s, sb):
                        """x_s = lin_w_s @ h_s (+lin_b) in z-major; BatchNorm;
                        transpose to batch-major; store y_out[s-1].  All inputs
                        ready when emitted -> runs inside the AllGather wait."""
                        gs = nc.gpsimd
                        for k in range(KC):
                            nc.tensor.matmul(xz[:], linwT_sb[:, k, :],
                                             h_TB[:, sb, k, :],
                                             start=(k == 0), stop=(k == KC - 1))
                        nc.scalar.activation(xf[:], xz[:], AF.Identity, bias=lb_sb[:])
                        nc.vector.tensor_reduce(stz[:, 0:1], xf[:],
                                                mybir.AxisListType.X, OP.add)
                        gs.tensor_tensor(sqz[:], xf[:], xf[:], OP.mult)
                        nc.vector.tensor_reduce(stz[:, 1:2], sqz[:],
                                                mybir.AxisListType.X, OP.add)
                        gs.tensor_scalar(stz[:, 2:3], stz[:, 0:1], 1.0 / B, None, OP.mult)
                        gs.tensor_scalar(stz[:, 3:4], stz[:, 1:2], 1.0 / B, None, OP.mult)
                        gs.tensor_tensor(stz[:, 4:5], stz[:, 2:3], stz[:, 2:3], OP.mult)
                        gs.tensor_tensor(stz[:, 4:5], stz[:, 3:4], stz[:, 4:5], OP.subtract)
                        gs.tensor_scalar(stz[:, 4:5], stz[:, 4:5], 1.0, EPS,
                                         OP.mult, OP.add)
                        emit_rsqrt(stz[:, 6:7], stz[:, 4:5])                    # rs
                        gs.tensor_tensor(stz[:, 7:8], stz[:, 2:3], stz[:, 6:7], OP.mult)
                        gs.tensor_scalar(stz[:, 7:8], stz[:, 7:8], -1.0, None, OP.mult)
                        nc.scalar.activation(yz[:], xf[:], AF.Identity,
                                             bias=stz[:, 7:8], scale=stz[:, 6:7])
                        for u2 in range(2):
                            nc.tensor.transpose(typ[u2], yz[:, u2 * BH:(u2 + 1) * BH],
                                                ident[0:ZS, 0:ZS])
                            nc.scalar.activation(ybm[u2][:], typ[u2], AF.Copy)
                            nc.scalar.dma_start(y_out.ap()[s - 1, u2 * BH:(u2 + 1) * BH, :],
                                                ybm[u2][:])

                    for s in range(2, TLIM + 1):  # steps 2..T, h_{s-1} -> h_s
                        for u in range(2):
                            sb = (s - 1) % 2
                            us = slice(u * BH, (u + 1) * BH)
                            # bias rows first (no h dependency -> PE head start)
                            nc.tensor.matmul(gbA[u][:], ones16[:], bias_row[0:1, 0:512],
                                             start=True, stop=False)
                            nc.tensor.matmul(gbB[u][:], ones16[:], bias_row[0:1, 512:1024],
                                             start=True, stop=False)
                            for k in range(KC):
                                lhs = h_TB[:, sb, k, us]
                                nc.tensor.matmul(gbA[u][:], lhs, W_all[:, k, 0:512],
                                                 start=False, stop=(k == KC - 1))
                                nc.tensor.matmul(gbB[u][:], lhs, W_all[:, k, 512:1024],
                                                 start=False, stop=(k == KC - 1))
                            # gates: psum layout gbA=[r|z], gbB=[in|hn] (+bias);
                            # elementwise split DVE (j=0) / GpSimd (j=1)
                            nc.scalar.activation(ru_sb[u][:], gbA[u][:], AF.Sigmoid)
                            for j in range(2):
                                js = slice(j * 128, j * 128 + 128)
                                hs2 = slice(HS + j * 128, HS + j * 128 + 128)
                                # psum-reading ops stay on DVE
                                nc.vector.tensor_tensor(e_sb[u][:, js], gbB[u][:, hs2],
                                                        ru_sb[u][:, js], OP.mult)
                                nc.vector.tensor_tensor(e_sb[u][:, js], e_sb[u][:, js],
                                                        gbB[u][:, js], OP.add)
                                nc.scalar.activation(d_sb[u][:, js], e_sb[u][:, js],
                                                     AF.Tanh)
                            for j, eng in ((0, nc.vector), (1, nc.gpsimd)):
                                js = slice(j * 128, j * 128 + 128)
                                hs2 = slice(HS + j * 128, HS + j * 128 + 128)
                                eng.tensor_tensor(e_sb[u][:, js], h_new[u][:, js],
                                                  d_sb[u][:, js], OP.subtract)
                                eng.tensor_tensor(e_sb[u][:, js], ru_sb[u][:, hs2],
                                                  e_sb[u][:, js], OP.mult)
                                eng.tensor_tensor(h_new[u][:, js], d_sb[u][:, js],
                                                  e_sb[u][:, js], OP.add)
                                nc.tensor.transpose(tp2[j], h_new[u][:, ts(j, 128)],
                                                    ident[:])
                                nc.scalar.activation(h_new_x[u][:, j, :], tp2[j], AF.Copy)
                            if s == TLIM:
                                # last step: both halves ride ONE fused AllGather
                                sync.dma_start(
                                    cc_fin.ap().rearrange("(j p) b -> p j b", p=128)
                                    [:, :, us],
                                    h_new_x[u][:, :, :])
                                if u == 1:
                                    nc.gpsimd.collective_compute(
                                        "AllGather", OP.bypass, replica_groups=rg,
                                        ins=[cc_fin.ap().opt()],
                                        outs=[cc_fout.ap().opt()])
                                    cco_r = cc_fout.ap().rearrange("(k p) b -> p k b", p=128)
                                    for v in range(2):
                                        bs = slice(v * BH, (v + 1) * BH)
                                        sync.dma_start(h_TB[:, s % 2, 0:KC // 2, bs],
                                                       cco_r[:, 0:KC // 2, bs])
                                        nc.scalar.dma_start(h_TB[:, s % 2, KC // 2:, bs],
                                                            cco_r[:, KC // 2:, bs])
                            else:
                                sync.dma_start(cc_in[u][s - 1].ap()
                                               .rearrange("(j p) b -> p j b", p=128),
                                               h_new_x[u][:, :, :])
                                nc.gpsimd.collective_compute(
                                    "AllGather", OP.bypass, replica_groups=rg,
                                    ins=[cc_in[u][s - 1].ap().opt()],
                                    outs=[cc_out[u][s - 1].ap().opt()])
                                cco_r = cc_out[u][s - 1].ap().rearrange("(k p) b -> p k b", p=128)
                                # chunk 0 first so the next kloop resumes ASAP
                                sync.dma_start(h_TB[:, s % 2, 0:2, us], cco_r[:, 0:2, :])
                                nc.scalar.dma_start(h_TB[:, s % 2, 2:6, us], cco_r[:, 2:6, :])
                                sync.dma_start(h_TB[:, s % 2, 6:11, us], cco_r[:, 6:11, :])
                                nc.scalar.dma_start(h_TB[:, s % 2, 11:KC, us], cco_r[:, 11:KC, :])
                            if NFILL:
                                fillers(sb, NFILL)
                        # x_{s-1} output pass rides the AllGather windows
                        proj_pass(s - 1, sb)

                    # tail: x_T
                    proj_pass(TLIM, TLIM % 2)

    nc.compile()
    return nc


_NC_CACHE = [None]


def kernel(z, fc_w, fc_b, fc_u, w_ih, w_hh, b_ih, b_hh, lin_w, lin_b):
    z = np.ascontiguousarray(np.asarray(z, dtype=np.float32))
    fc_w = np.ascontiguousarray(np.asarray(fc_w, dtype=np.float32))
    fc_b = np.asarray(fc_b, dtype=np.float32)
    fc_u = np.asarray(fc_u, dtype=np.float32)
    w_ih = np.asarray(w_ih, dtype=np.float32)
    w_hh = np.asarray(w_hh, dtype=np.float32)
    b_ih = np.asarray(b_ih, dtype=np.float32)
    b_hh = np.asarray(b_hh, dtype=np.float32)
    lin_w = np.asarray(lin_w, dtype=np.float32)
    lin_b = np.asarray(lin_b, dtype=np.float32)

    zT = np.ascontiguousarray(z.T)
    fc_wT = np.ascontiguousarray(fc_w.T)
    lin_wT = np.ascontiguousarray(lin_w.T)
    w_ih4 = w_ih.reshape(3, NC, HS, Z)
    w_hh4 = w_hh.reshape(3, NC, HS, H)
    b_ih3 = b_ih.reshape(3, NC, HS)
    b_hh3 = b_hh.reshape(3, NC, HS)

    in_maps = []
    for c in range(NC):
        wihs = w_ih4[:, c].reshape(GR, Z)
        whhs = w_hh4[:, c].reshape(GR, H)
        in_maps.append({
            "zT": zT,
            "fc_w": fc_w,
            "fc_wT": fc_wT,
            "fc_u": fc_u,
            "fc_b": fc_b,
            "w_ihT_s": np.ascontiguousarray(wihs.T),
            "w_hhT_s": np.ascontiguousarray(whhs.T).astype(np.float16),
            "lin_w": lin_w,
            "lin_wT_s": np.ascontiguousarray(lin_wT[:, c * ZS:(c + 1) * ZS]).astype(np.float16),
            "lin_b": lin_b,
            "lin_b_s": np.ascontiguousarray(lin_b[c * ZS:(c + 1) * ZS]),
            "b_ih_s": np.ascontiguousarray(b_ih3[:, c].reshape(GR)),
            "b_hh_s": np.ascontiguousarray(b_hh3[:, c].reshape(GR)),
        })

    if _NC_CACHE[0] is None:
        _NC_CACHE[0] = build_nc()
    nc = _NC_CACHE[0]

    trace = os.environ.get("BASS_KERNEL_TRACE") == "1"
    if trace:
        _install_ntff_hook()
    res = run_bass_kernel_spmd(nc, in_maps, core_ids=list(range(NC)), trace=trace)
    LAST_EXEC_NS[0] = res.exec_time_ns
    LAST_RESULTS[0] = res

    full = np.empty((T, B, Z), dtype=np.float32)
    for c in range(NC):
        full[:, :, c * ZS:(c + 1) * ZS] = res.results[c]["y_part"]
    return full.transpose(1, 0, 2).reshape(B * T, Z)
